# revision 13
# baseline (speedup 1.0000x reference)
"""Trainium2 Bass kernel for nn_Compression, v5.

Computes: out = X + GAMMA * (P @ (P.T @ X)),  P = softmax(X @ W.T + b)

Strategy (8 NeuronCores, data-parallel over N):
  Phase A (per row-tile of 128 rows, software-pipelined):
    - DMA X (f32, two tiles per DMA, kept resident for the residual);
      DVE casts each tile to fp8e4 (the correction term is GAMMA-scaled,
      so fp8's ~6% relative error contributes only ~1e-5 to the output).
    - X.T via PE transposes of the fp8 tile VIEWED AS fp16 (adjacent-d
      pairs travel together): 4 transposes per tile instead of 8. The
      PSUM drain de-interleaves the pairs into the standard DoubleRow
      weight layout; paired with Wq2[p,q,i,c] = W.T[256q+2p+i, c].
    - Logits via 4 fp8 DoubleRow matmuls (K=256 each) + bf16 bias matmul.
    - Softmax: ACT exp with row-sum accumulator, DVE reciprocal + scale
      casting P straight to fp8.
    - P.T @ X accumulated into 4 resident PSUM banks via fp8 DoubleRow
      matmuls over row-tile PAIRS (K=256 = two row tiles per matmul).
    - P.T (phase B lhsT) via fp8 PE transposes (stride-2 out), ACT drain.
  - PtX partials drain to bf16; a tiny warm-up AllReduce issued at t=0
    absorbs the collectives stream's one-time barrier (~50us); the real
    AllReduce runs in TWO D-halves (bf16, 256 KiB each) so phase B on
    half 0 overlaps the half-1 collective.
  Phase B (per row-tile pair, per D-half):
    - corr = P @ (gamma * PtX) with ONE fp8 DoubleRow matmul per tile
      per half (lhsT = resident fp8 P.T, K=256=C).
    - ACT drains PSUM to bf16, DVE adds the exact f32 residual writing
      bf16, one store DMA per tile PAIR; the host upcasts to f32
      (output values are bf16-rounded: ~1e-3 rel err vs the 2e-2 gate).

The host side only reshapes: shards X rows, passes W transposed (pure
relayout, still f32) and b as-is.
"""

import sys

import numpy as np

if "/opt/trn_rl_repo" not in sys.path:
    sys.path.insert(0, "/opt/trn_rl_repo")

N, D, C = 32768, 1024, 256
GAMMA = 1e-4
NCORES = 8
NLOC = N // NCORES  # 4096
P = 128
NT = NLOC // P  # 32
NPAIR = NT // 2  # 16
DH = 512

_cache = {}


def _build_nc():
    import concourse.tile as tile
    from concourse import bacc
    import concourse.mybir as mybir
    from concourse.masks import make_identity
    from contextlib import ExitStack

    f32 = mybir.dt.float32
    bf16 = mybir.dt.bfloat16
    fp8 = mybir.dt.float8e4
    f16 = mybir.dt.float16
    DR = mybir.MatmulPerfMode.DoubleRow
    AF = mybir.ActivationFunctionType

    nc = bacc.Bacc("TRN2", target_bir_lowering=False, debug=False, num_devices=NCORES)
    X = nc.dram_tensor("X", [NLOC, D], f32, kind="ExternalInput").ap()
    Wt = nc.dram_tensor("Wt", [D, C], f32, kind="ExternalInput").ap()
    bvec = nc.dram_tensor("b", [C], f32, kind="ExternalInput").ap()
    out = nc.dram_tensor("out", [NLOC, D], bf16, kind="ExternalOutput").ap()

    with tile.TileContext(nc) as tc, ExitStack() as ctx:
        const = ctx.enter_context(tc.tile_pool(name="const", bufs=1))
        xres = ctx.enter_context(tc.tile_pool(name="xres", bufs=1))
        # xqp holds an fp8 row-tile PAIR; written at load(2p)/load(2p+1),
        # read by transpose and by ptx(pair) ~3 steps later.
        xqp = ctx.enter_context(tc.tile_pool(name="xqp", bufs=3))
        xtp = ctx.enter_context(tc.tile_pool(name="xtp", bufs=2))
        ppool = ctx.enter_context(tc.tile_pool(name="ppool", bufs=4))
        pqp = ctx.enter_context(tc.tile_pool(name="pqp", bufs=3))
        spool = ctx.enter_context(tc.tile_pool(name="spool", bufs=4))
        opool = ctx.enter_context(tc.tile_pool(name="opool", bufs=4))
        cpool = ctx.enter_context(tc.tile_pool(name="cpool", bufs=4))
        dram = ctx.enter_context(tc.tile_pool(name="dram", bufs=1, space="DRAM"))

        Xall = xres.tile([P, NT, D], f32)  # resident f32 X (residual)
        Pt = const.tile([P, 2, NLOC], fp8)  # resident P.T (c-chunk major)

        # X loads first: two row-tiles per DMA so the Sync queue issues
        # 16 ops, and the first tiles land before the W/bias setup DMAs.
        xr = X.rearrange("(t p) d -> p t d", p=P)
        for t in range(NPAIR):
            nc.sync.dma_start(Xall[:, 2 * t:2 * t + 2, :], xr[:, 2 * t:2 * t + 2, :])

        ident8 = const.tile([P, P], fp8)
        make_identity(nc, ident8)
        ident16 = const.tile([P, P], f16)
        make_identity(nc, ident16)

        # Wq2[p, q, i, c] = W.T[256q + 2p + i, c] in fp8: the DoubleRow
        # K-pair (p, i) matches the fp16-packed transpose output.
        Wq2 = const.tile([P, 4, 2, C], fp8)
        with tc.tile_pool(name="wtmp", bufs=1) as wtmp:
            wt_f = wtmp.tile([P, 4, 2, C], f32)
            nc.sync.dma_start(
                wt_f[:], Wt.rearrange("(q p two) c -> p q two c", p=P, two=2)
            )
            nc.scalar.copy(Wq2[:], wt_f[:])

        ones1 = const.tile([1, P], bf16)
        nc.vector.memset(ones1[:], 1.0)
        b_sb = const.tile([1, C], bf16)
        with tc.tile_pool(name="btmp", bufs=1) as btmp:
            b_f = btmp.tile([1, C], f32)
            nc.sync.dma_start(b_f[:], bvec.rearrange("(o c) -> o c", o=1))
            nc.vector.tensor_copy(b_sb[:], b_f[:])

        ar_in = [dram.tile([C, DH], bf16, name=f"ar_in{h}") for h in range(2)]
        ar_out = [
            dram.tile([C, DH], bf16, addr_space="Shared", name=f"ar_out{h}")
            for h in range(2)
        ]

        # Tiny warm-up AllReduce: absorbs the collectives stream's
        # one-time BARRIER/init concurrently with phase A.
        warm_in = dram.tile([1, 64], f32, name="warm_in")
        warm_out = dram.tile([1, 64], f32, addr_space="Shared", name="warm_out")
        with tc.tile_pool(name="wrm", bufs=1) as wrm:
            w_sb = wrm.tile([1, 64], f32)
            nc.vector.memset(w_sb[:], 0.0)
            nc.sync.dma_start(warm_in[:], w_sb[:])
        nc.gpsimd.collective_compute(
            "AllReduce",
            mybir.AluOpType.add,
            replica_groups=[list(range(NCORES))],
            ins=[warm_in[:].opt()],
            outs=[warm_out[:].opt()],
        )

        # ---- phase A ----
        def s_cast(i):
            if i % 2 == 0:
                return xqp.tile([P, 2, D], fp8, name="xq", tag="xq")
            return None

        def s_transpose(i, xq):
            # fp8 cast on DVE, then 4 PE transposes of the fp16 view
            # (adjacent-d fp8 pairs); the drain de-interleaves into the
            # standard DR weight layout, alternating ACT/DVE.
            nc.vector.tensor_copy(xq[:, i % 2, :], Xall[:, i, :])
            xt2 = xtp.tile([P, 4, 2, P], fp8, name="xt", tag="xt")
            trp = psA.tile([P, 4, P], f16, name="trp", tag="trp")
            xq16 = xq[:, i % 2, :].bitcast(f16)  # [P, 512]
            for q in range(4):
                nc.tensor.matmul(
                    trp[:, q, :],
                    xq16[:, q * P:(q + 1) * P],
                    ident16[:],
                    is_transpose=True,
                    start=(q == 0),
                    stop=(q == 3),
                )
            src = trp[:].bitcast(fp8).rearrange("p q (n two) -> p q two n", two=2)
            if i % 2 == 0:
                nc.scalar.copy(xt2[:], src)
            else:
                nc.vector.tensor_copy(xt2[:], src)
            return xt2

        def s_logits(i, xt2):
            lg = psL.tile([P, C], f32, name="lg", tag="lg")
            for q in range(4):
                nc.tensor.matmul(
                    lg[:],
                    xt2[:, q, :, :],
                    Wq2[:, q, :, :],
                    start=(q == 0),
                    stop=False,
                    perf_mode=DR,
                )
            nc.tensor.matmul(lg[:], ones1[:], b_sb[:], start=False, stop=True)
            return lg

        def s_softmax(i, lg, pq):
            # |logits| <= ~10 so exp is safe without max-subtraction
            p_sb = ppool.tile([P, C], f32, name="p_sb", tag="p")
            ssum = spool.tile([P, 1], f32, name="ssum", tag="s")
            nc.scalar.activation(p_sb[:], lg[:], AF.Exp, accum_out=ssum[:])
            rinv = spool.tile([P, 1], f32, name="rinv", tag="r")
            nc.vector.reciprocal(rinv[:], ssum[:])
            nc.vector.tensor_scalar_mul(pq[:, i % 2, :], p_sb[:], rinv[:])

        def s_ptx(pair, pq, xq):
            # PtX += P_pair.T @ X_pair, fp8 DoubleRow over K=256 rows
            for cc in range(2):
                for h in range(2):
                    nc.tensor.matmul(
                        ptx_ps[2 * cc + h][:],
                        pq[:, :, cc * P:(cc + 1) * P],
                        xq[:, :, h * DH:(h + 1) * DH],
                        start=(pair == 0),
                        stop=(pair == NPAIR - 1),
                        perf_mode=DR,
                    )
            # P.T for phase B: 4 fp8 transposes (stride-2 out), ACT drain.
            ptp = psP.tile([P, 2, C, 2], fp8, name="ptp", tag="ptp")
            for j in range(2):
                for cc in range(2):
                    nc.tensor.matmul(
                        ptp[:, j, cc * P:(cc + 1) * P, 0],
                        pq[:, j, cc * P:(cc + 1) * P],
                        ident8[:],
                        is_transpose=True,
                        start=(j == 0 and cc == 0),
                        stop=(j == 1 and cc == 1),
                    )
            dst = Pt[:, :, 2 * pair * P:(2 * pair + 2) * P].rearrange(
                "p cc (j r) -> p cc j r", j=2
            )
            src = ptp[:, :, :, 0].rearrange("p j (cc r) -> p cc j r", cc=2)
            nc.scalar.copy(dst, src)

        with tc.tile_pool(name="psA", bufs=2, space="PSUM") as psA, \
             tc.tile_pool(name="psP", bufs=1, space="PSUM") as psP, \
             tc.tile_pool(name="psL", bufs=1, space="PSUM") as psL, \
             tc.tile_pool(name="psX", bufs=1, space="PSUM") as psX:
            ptx_ps = [
                psX.tile([P, DH], f32, name=f"ptx_{c}_{h}", tag=f"ptx_{c}_{h}")
                for c in range(2)
                for h in range(2)
            ]
            # Pipeline: at step i run logits(i), transpose(i+1), and ptx
            # for the pair ending at tile i-2. The 2-step skew lets the
            # ACT exp latency hide under PE work.
            state = {}

            xq0 = s_cast(0)
            state[0] = (xq0, s_transpose(0, xq0), None)
            xq1 = s_cast(1)
            state[1] = (xq1 if xq1 is not None else xq0, None, None)
            for i in range(NT):
                xq_i, xt_i, _ = state[i]
                lg = s_logits(i, xt_i)
                if i % 2 == 0:
                    pq = pqp.tile([P, 2, C], fp8, name="pq", tag="pq")
                else:
                    pq = state[i - 1][2]
                s_softmax(i, lg, pq)
                state[i] = (xq_i, xt_i, pq)
                if i + 1 < NT:
                    xq_n, _, _ = state[i + 1]
                    state[i + 1] = (xq_n, s_transpose(i + 1, xq_n), None)
                if i + 2 < NT:
                    xq_f = s_cast(i + 2)
                    state[i + 2] = (
                        xq_f if xq_f is not None else state[i + 1][0],
                        None,
                        None,
                    )
                if i >= 3 and (i - 2) % 2 == 1:
                    pair = (i - 3) // 2
                    xq_p = state.pop(2 * pair)[0]
                    pq_p = state.pop(2 * pair + 1)[2]
                    s_ptx(pair, pq_p, xq_p)
            # in-loop ptx covers pairs 0..NPAIR-2; finish the last pair
            for pair in (NPAIR - 1,):
                xq_p = state.pop(2 * pair)[0]
                pq_p = state.pop(2 * pair + 1)[2]
                s_ptx(pair, pq_p, xq_p)

            # PSUM -> SBUF -> DRAM bounce, one per D-half, cast to bf16
            # (the PtX rounding is gamma-scaled: invisible in the output).
            for h in range(2):
                s = const.tile([P, 2, DH], bf16, name=f"stg{h}", tag=f"stg{h}")
                nc.vector.tensor_copy(s[:, 0, :], ptx_ps[h][:])
                nc.scalar.copy(s[:, 1, :], ptx_ps[2 + h][:])
                nc.sync.dma_start(
                    ar_in[h].rearrange("(c p) d -> p c d", p=P), s[:]
                )

        # ---- phase B, interleaved with the collectives: AllReduce h=1 is
        # emitted AFTER phase B h=0 so h=0's consumers only wait on the
        # first collective's completion tick, and the second collective
        # runs concurrently with h=0 compute. gamma folded into PtX. ----
        def ar(h):
            nc.gpsimd.collective_compute(
                "AllReduce",
                mybir.AluOpType.add,
                replica_groups=[list(range(NCORES))],
                ins=[ar_in[h][:].opt()],
                outs=[ar_out[h][:].opt()],
            )

        def phase_b(h, psB):
            pall = const.tile([P, 2, DH], bf16, name=f"pall{h}", tag=f"stg{h}")
            nc.sync.dma_start(
                pall[:], ar_out[h].rearrange("(c p) d -> p c d", p=P)
            )
            ptxq = const.tile([P, 2, DH], fp8, name=f"ptxq{h}")
            nc.vector.tensor_scalar_mul(ptxq[:], pall[:], GAMMA)
            o_dst = out[:, h * DH:(h + 1) * DH].rearrange("(t p) d -> p t d", p=P)
            for t in range(NPAIR):
                o2 = opool.tile([P, 2, DH], bf16, name="o2", tag="o")
                for j in range(2):
                    i = 2 * t + j
                    cor = psB.tile([P, DH], f32, name="cor", tag="cor")
                    nc.tensor.matmul(
                        cor[:],
                        Pt[:, :, i * P:(i + 1) * P],
                        ptxq[:],
                        start=True,
                        stop=True,
                        perf_mode=DR,
                    )
                    cors = cpool.tile([P, DH], bf16, name="cors", tag="cs")
                    nc.scalar.copy(cors[:], cor[:])
                    nc.vector.tensor_add(
                        o2[:, j, :], cors[:], Xall[:, i, h * DH:(h + 1) * DH]
                    )
                nc.sync.dma_start(o_dst[:, 2 * t:2 * t + 2, :], o2[:])

        with tc.tile_pool(name="psB", bufs=8, space="PSUM") as psB:
            ar(0)
            phase_b(0, psB)
            ar(1)
            phase_b(1, psB)

    nc.finalize()
    return nc


def _run(inputs, trace=False, **kwargs):
    from concourse import bass_utils

    if "nc" not in _cache:
        _cache["nc"] = _build_nc()
    nc = _cache["nc"]

    X = np.ascontiguousarray(np.asarray(inputs["X"], dtype=np.float32))
    W = np.ascontiguousarray(np.asarray(inputs["W"], dtype=np.float32))
    b = np.ascontiguousarray(np.asarray(inputs["b"], dtype=np.float32))
    Wt = np.ascontiguousarray(W.T)

    in_maps = [
        {"X": X[i * NLOC:(i + 1) * NLOC], "Wt": Wt, "b": b} for i in range(NCORES)
    ]
    res = bass_utils.run_bass_kernel_spmd(
        nc, in_maps, core_ids=list(range(NCORES)), trace=trace, **kwargs
    )
    outp = np.concatenate(
        [np.asarray(res.results[i]["out"]).astype(np.float32) for i in range(NCORES)],
        axis=0,
    )
    return outp, res


def kernel(**inputs):
    outp, _ = _run(inputs, trace=False)
    return outp


# revision 15
# speedup vs baseline: 1.3061x; 1.3061x over previous
"""Trainium2 Bass kernel for nn_Compression, v5.

Computes: out = X + GAMMA * (P @ (P.T @ X)),  P = softmax(X @ W.T + b)

Strategy (8 NeuronCores, data-parallel over N):
  Phase A (per row-tile of 128 rows, software-pipelined):
    - DMA X (f32, two tiles per DMA, kept resident for the residual);
      DVE casts each tile to fp8e4 (the correction term is GAMMA-scaled,
      so fp8's ~6% relative error contributes only ~1e-5 to the output).
    - X.T via PE transposes of the fp8 tile VIEWED AS fp16 (adjacent-d
      pairs travel together): 4 transposes per tile instead of 8. The
      PSUM drain de-interleaves the pairs into the standard DoubleRow
      weight layout; paired with Wq2[p,q,i,c] = W.T[256q+2p+i, c].
    - Logits via 4 fp8 DoubleRow matmuls (K=256 each) + bf16 bias matmul.
    - Softmax: ACT exp with row-sum accumulator, DVE reciprocal + scale
      casting P straight to fp8.
    - P.T @ X accumulated into 4 resident PSUM banks via fp8 DoubleRow
      matmuls over row-tile PAIRS (K=256 = two row tiles per matmul).
    - P.T (phase B lhsT) via fp8 PE transposes (stride-2 out), ACT drain.
  - PtX partials drain to bf16; a tiny warm-up AllReduce issued at t=0
    absorbs the collectives stream's one-time barrier (~50us); the real
    AllReduce runs in TWO D-halves (bf16, 256 KiB each) so phase B on
    half 0 overlaps the half-1 collective.
  Phase B (per row-tile pair, per D-half):
    - corr = P @ (gamma * PtX) with ONE fp8 DoubleRow matmul per tile
      per half (lhsT = resident fp8 P.T, K=256=C).
    - ACT drains PSUM to bf16, DVE adds the exact f32 residual writing
      bf16, one store DMA per tile PAIR; the host upcasts to f32
      (output values are bf16-rounded: ~1e-3 rel err vs the 2e-2 gate).

The host side only reshapes: shards X rows, passes W transposed (pure
relayout, still f32) and b as-is.
"""

import sys

import numpy as np

if "/opt/trn_rl_repo" not in sys.path:
    sys.path.insert(0, "/opt/trn_rl_repo")

N, D, C = 32768, 1024, 256
GAMMA = 1e-4
NCORES = 8
NLOC = N // NCORES  # 4096
P = 128
NT = NLOC // P  # 32
NPAIR = NT // 2  # 16
DH = 512

_cache = {}


def _build_nc():
    import concourse.tile as tile
    from concourse import bacc
    import concourse.mybir as mybir
    from concourse.masks import make_identity
    from contextlib import ExitStack

    f32 = mybir.dt.float32
    bf16 = mybir.dt.bfloat16
    fp8 = mybir.dt.float8e4
    f16 = mybir.dt.float16
    DR = mybir.MatmulPerfMode.DoubleRow
    AF = mybir.ActivationFunctionType

    nc = bacc.Bacc("TRN2", target_bir_lowering=False, debug=False, num_devices=NCORES)
    X = nc.dram_tensor("X", [NLOC, D], f32, kind="ExternalInput").ap()
    Wt = nc.dram_tensor("Wt", [D, C], f32, kind="ExternalInput").ap()
    bvec = nc.dram_tensor("b", [C], f32, kind="ExternalInput").ap()
    out = nc.dram_tensor("out", [NLOC, D], bf16, kind="ExternalOutput").ap()

    with tile.TileContext(nc) as tc, ExitStack() as ctx:
        const = ctx.enter_context(tc.tile_pool(name="const", bufs=1))
        xres = ctx.enter_context(tc.tile_pool(name="xres", bufs=1))
        # xqp holds an fp8 row-tile PAIR; written at load(2p)/load(2p+1),
        # read by transpose and by ptx(pair) ~3 steps later.
        xqp = ctx.enter_context(tc.tile_pool(name="xqp", bufs=3))
        xtp = ctx.enter_context(tc.tile_pool(name="xtp", bufs=2))
        ppool = ctx.enter_context(tc.tile_pool(name="ppool", bufs=4))
        pqp = ctx.enter_context(tc.tile_pool(name="pqp", bufs=3))
        spool = ctx.enter_context(tc.tile_pool(name="spool", bufs=4))
        opool = ctx.enter_context(tc.tile_pool(name="opool", bufs=4))
        cpool = ctx.enter_context(tc.tile_pool(name="cpool", bufs=4))
        dram = ctx.enter_context(tc.tile_pool(name="dram", bufs=1, space="DRAM"))

        Xall = xres.tile([P, NT, D], f32)  # resident f32 X (residual)
        Pt = const.tile([P, 2, NLOC], fp8)  # resident P.T (c-chunk major)

        ident8 = const.tile([P, P], fp8)
        make_identity(nc, ident8)
        ident16 = const.tile([P, P], f16)
        make_identity(nc, ident16)

        # Wq2[p, q, i, c] = W.T[256q + 2p + i, c] in fp8: the DoubleRow
        # K-pair (p, i) matches the fp16-packed transpose output.
        Wq2 = const.tile([P, 4, 2, C], fp8)
        with tc.tile_pool(name="wtmp", bufs=1) as wtmp:
            wt_f = wtmp.tile([P, 4, 2, C], f32)
            nc.sync.dma_start(
                wt_f[:], Wt.rearrange("(q p two) c -> p q two c", p=P, two=2)
            )
            nc.scalar.copy(Wq2[:], wt_f[:])

        ones1 = const.tile([1, P], bf16)
        nc.vector.memset(ones1[:], 1.0)
        b_sb = const.tile([1, C], bf16)
        with tc.tile_pool(name="btmp", bufs=1) as btmp:
            b_f = btmp.tile([1, C], f32)
            nc.sync.dma_start(b_f[:], bvec.rearrange("(o c) -> o c", o=1))
            nc.vector.tensor_copy(b_sb[:], b_f[:])

        ar_in = [dram.tile([C, DH], bf16, name=f"ar_in{h}") for h in range(2)]
        ar_out = [
            dram.tile([C, DH], bf16, addr_space="Shared", name=f"ar_out{h}")
            for h in range(2)
        ]

        # Tiny warm-up AllReduce: absorbs the collectives stream's
        # one-time BARRIER/init concurrently with phase A.
        warm_in = dram.tile([1, 64], f32, name="warm_in")
        warm_out = dram.tile([1, 64], f32, addr_space="Shared", name="warm_out")
        with tc.tile_pool(name="wrm", bufs=1) as wrm:
            w_sb = wrm.tile([1, 64], f32)
            nc.vector.memset(w_sb[:], 0.0)
            nc.sync.dma_start(warm_in[:], w_sb[:])
        nc.gpsimd.collective_compute(
            "AllReduce",
            mybir.AluOpType.add,
            replica_groups=[list(range(NCORES))],
            ins=[warm_in[:].opt()],
            outs=[warm_out[:].opt()],
        )

        # X loads AFTER the small setup DMAs (the Sync queue is in-order:
        # W/b must not queue behind 16 MB of X traffic); two row-tiles
        # per DMA halves the issue count.
        xr = X.rearrange("(t p) d -> p t d", p=P)
        for t in range(NPAIR):
            nc.sync.dma_start(Xall[:, 2 * t:2 * t + 2, :], xr[:, 2 * t:2 * t + 2, :])

        # ---- phase A ----
        def s_cast(i):
            if i % 2 == 0:
                return xqp.tile([P, 2, D], fp8, name="xq", tag="xq")
            return None

        def s_transpose(i, xq):
            # fp8 cast on DVE, then 4 PE transposes of the fp16 view
            # (adjacent-d fp8 pairs); the drain de-interleaves into the
            # standard DR weight layout, alternating ACT/DVE.
            nc.vector.tensor_copy(xq[:, i % 2, :], Xall[:, i, :])
            xt2 = xtp.tile([P, 4, 2, P], fp8, name="xt", tag="xt")
            trp = psA.tile([P, 4, P], f16, name="trp", tag="trp")
            xq16 = xq[:, i % 2, :].bitcast(f16)  # [P, 512]
            for q in range(4):
                nc.tensor.matmul(
                    trp[:, q, :],
                    xq16[:, q * P:(q + 1) * P],
                    ident16[:],
                    is_transpose=True,
                    start=(q == 0),
                    stop=(q == 3),
                )
            src = trp[:].bitcast(fp8).rearrange("p q (n two) -> p q two n", two=2)
            if i % 2 == 0:
                nc.scalar.copy(xt2[:], src)
            else:
                nc.vector.tensor_copy(xt2[:], src)
            return xt2

        def s_logits(i, xt2):
            lg = psL.tile([P, C], f32, name="lg", tag="lg")
            for q in range(4):
                nc.tensor.matmul(
                    lg[:],
                    xt2[:, q, :, :],
                    Wq2[:, q, :, :],
                    start=(q == 0),
                    stop=False,
                    perf_mode=DR,
                )
            nc.tensor.matmul(lg[:], ones1[:], b_sb[:], start=False, stop=True)
            return lg

        def s_softmax(i, lg, pq):
            # |logits| <= ~10 so exp is safe without max-subtraction
            p_sb = ppool.tile([P, C], f32, name="p_sb", tag="p")
            ssum = spool.tile([P, 1], f32, name="ssum", tag="s")
            nc.scalar.activation(p_sb[:], lg[:], AF.Exp, accum_out=ssum[:])
            rinv = spool.tile([P, 1], f32, name="rinv", tag="r")
            nc.vector.reciprocal(rinv[:], ssum[:])
            nc.vector.tensor_scalar_mul(pq[:, i % 2, :], p_sb[:], rinv[:])

        def s_ptx(pair, pq, xq):
            # PtX += P_pair.T @ X_pair, fp8 DoubleRow over K=256 rows
            for cc in range(2):
                for h in range(2):
                    nc.tensor.matmul(
                        ptx_ps[2 * cc + h][:],
                        pq[:, :, cc * P:(cc + 1) * P],
                        xq[:, :, h * DH:(h + 1) * DH],
                        start=(pair == 0),
                        stop=(pair == NPAIR - 1),
                        perf_mode=DR,
                    )
            # P.T for phase B: 4 fp8 transposes (stride-2 out), ACT drain.
            ptp = psP.tile([P, 2, C, 2], fp8, name="ptp", tag="ptp")
            for j in range(2):
                for cc in range(2):
                    nc.tensor.matmul(
                        ptp[:, j, cc * P:(cc + 1) * P, 0],
                        pq[:, j, cc * P:(cc + 1) * P],
                        ident8[:],
                        is_transpose=True,
                        start=(j == 0 and cc == 0),
                        stop=(j == 1 and cc == 1),
                    )
            dst = Pt[:, :, 2 * pair * P:(2 * pair + 2) * P].rearrange(
                "p cc (j r) -> p cc j r", j=2
            )
            src = ptp[:, :, :, 0].rearrange("p j (cc r) -> p cc j r", cc=2)
            nc.scalar.copy(dst, src)

        with tc.tile_pool(name="psA", bufs=2, space="PSUM") as psA, \
             tc.tile_pool(name="psP", bufs=1, space="PSUM") as psP, \
             tc.tile_pool(name="psL", bufs=1, space="PSUM") as psL, \
             tc.tile_pool(name="psX", bufs=1, space="PSUM") as psX:
            ptx_ps = [
                psX.tile([P, DH], f32, name=f"ptx_{c}_{h}", tag=f"ptx_{c}_{h}")
                for c in range(2)
                for h in range(2)
            ]
            # Pipeline: at step i run logits(i), transpose(i+1), and ptx
            # for the pair ending at tile i-2. The 2-step skew lets the
            # ACT exp latency hide under PE work.
            state = {}

            xq0 = s_cast(0)
            state[0] = (xq0, s_transpose(0, xq0), None)
            xq1 = s_cast(1)
            state[1] = (xq1 if xq1 is not None else xq0, None, None)
            for i in range(NT):
                xq_i, xt_i, _ = state[i]
                lg = s_logits(i, xt_i)
                if i % 2 == 0:
                    pq = pqp.tile([P, 2, C], fp8, name="pq", tag="pq")
                else:
                    pq = state[i - 1][2]
                s_softmax(i, lg, pq)
                state[i] = (xq_i, xt_i, pq)
                if i + 1 < NT:
                    xq_n, _, _ = state[i + 1]
                    state[i + 1] = (xq_n, s_transpose(i + 1, xq_n), None)
                if i + 2 < NT:
                    xq_f = s_cast(i + 2)
                    state[i + 2] = (
                        xq_f if xq_f is not None else state[i + 1][0],
                        None,
                        None,
                    )
                if i >= 3 and (i - 2) % 2 == 1:
                    pair = (i - 3) // 2
                    xq_p = state.pop(2 * pair)[0]
                    pq_p = state.pop(2 * pair + 1)[2]
                    s_ptx(pair, pq_p, xq_p)
            # in-loop ptx covers pairs 0..NPAIR-2; finish the last pair
            for pair in (NPAIR - 1,):
                xq_p = state.pop(2 * pair)[0]
                pq_p = state.pop(2 * pair + 1)[2]
                s_ptx(pair, pq_p, xq_p)

            # PSUM -> SBUF -> DRAM bounce, one per D-half, cast to bf16
            # (the PtX rounding is gamma-scaled: invisible in the output).
            for h in range(2):
                s = const.tile([P, 2, DH], bf16, name=f"stg{h}", tag=f"stg{h}")
                nc.vector.tensor_copy(s[:, 0, :], ptx_ps[h][:])
                nc.scalar.copy(s[:, 1, :], ptx_ps[2 + h][:])
                nc.sync.dma_start(
                    ar_in[h].rearrange("(c p) d -> p c d", p=P), s[:]
                )

        # ---- phase B, interleaved with the collectives: AllReduce h=1 is
        # emitted AFTER phase B h=0 so h=0's consumers only wait on the
        # first collective's completion tick, and the second collective
        # runs concurrently with h=0 compute. gamma folded into PtX. ----
        def ar(h):
            nc.gpsimd.collective_compute(
                "AllReduce",
                mybir.AluOpType.add,
                replica_groups=[list(range(NCORES))],
                ins=[ar_in[h][:].opt()],
                outs=[ar_out[h][:].opt()],
            )

        def phase_b(h, psB):
            pall = const.tile([P, 2, DH], bf16, name=f"pall{h}", tag=f"stg{h}")
            nc.sync.dma_start(
                pall[:], ar_out[h].rearrange("(c p) d -> p c d", p=P)
            )
            ptxq = const.tile([P, 2, DH], fp8, name=f"ptxq{h}")
            nc.vector.tensor_scalar_mul(ptxq[:], pall[:], GAMMA)
            o_dst = out[:, h * DH:(h + 1) * DH].rearrange("(t p) d -> p t d", p=P)
            for t in range(NPAIR):
                o2 = opool.tile([P, 2, DH], bf16, name="o2", tag="o")
                for j in range(2):
                    i = 2 * t + j
                    cor = psB.tile([P, DH], f32, name="cor", tag="cor")
                    nc.tensor.matmul(
                        cor[:],
                        Pt[:, :, i * P:(i + 1) * P],
                        ptxq[:],
                        start=True,
                        stop=True,
                        perf_mode=DR,
                    )
                    cors = cpool.tile([P, DH], bf16, name="cors", tag="cs")
                    nc.scalar.copy(cors[:], cor[:])
                    nc.vector.tensor_add(
                        o2[:, j, :], cors[:], Xall[:, i, h * DH:(h + 1) * DH]
                    )
                nc.sync.dma_start(o_dst[:, 2 * t:2 * t + 2, :], o2[:])

        with tc.tile_pool(name="psB", bufs=8, space="PSUM") as psB:
            ar(0)
            phase_b(0, psB)
            ar(1)
            phase_b(1, psB)

    nc.finalize()
    return nc


def _run(inputs, trace=False, **kwargs):
    from concourse import bass_utils

    if "nc" not in _cache:
        _cache["nc"] = _build_nc()
    nc = _cache["nc"]

    X = np.ascontiguousarray(np.asarray(inputs["X"], dtype=np.float32))
    W = np.ascontiguousarray(np.asarray(inputs["W"], dtype=np.float32))
    b = np.ascontiguousarray(np.asarray(inputs["b"], dtype=np.float32))
    Wt = np.ascontiguousarray(W.T)

    in_maps = [
        {"X": X[i * NLOC:(i + 1) * NLOC], "Wt": Wt, "b": b} for i in range(NCORES)
    ]
    res = bass_utils.run_bass_kernel_spmd(
        nc, in_maps, core_ids=list(range(NCORES)), trace=trace, **kwargs
    )
    outp = np.concatenate(
        [np.asarray(res.results[i]["out"]).astype(np.float32) for i in range(NCORES)],
        axis=0,
    )
    return outp, res


def kernel(**inputs):
    outp, _ = _run(inputs, trace=False)
    return outp


# revision 18
# speedup vs baseline: 1.3523x; 1.0353x over previous
"""Trainium2 Bass kernel for nn_Compression, v5.

Computes: out = X + GAMMA * (P @ (P.T @ X)),  P = softmax(X @ W.T + b)

Strategy (8 NeuronCores, data-parallel over N):
  Phase A (per row-tile of 128 rows, software-pipelined):
    - DMA X (f32, two tiles per DMA, kept resident for the residual);
      DVE casts each tile to fp8e4 (the correction term is GAMMA-scaled,
      so fp8's ~6% relative error contributes only ~1e-5 to the output).
    - X.T via PE transposes of the fp8 tile VIEWED AS fp16 (adjacent-d
      pairs travel together): 4 transposes per tile instead of 8. The
      PSUM drain de-interleaves the pairs into the standard DoubleRow
      weight layout; paired with Wq2[p,q,i,c] = W.T[256q+2p+i, c].
    - Logits via 4 fp8 DoubleRow matmuls (K=256 each) + bf16 bias matmul.
    - Softmax: ACT exp with row-sum accumulator, DVE reciprocal + scale
      casting P straight to fp8.
    - P.T @ X accumulated into 4 resident PSUM banks via fp8 DoubleRow
      matmuls over row-tile PAIRS (K=256 = two row tiles per matmul).
    - P.T (phase B lhsT) via fp8 PE transposes (stride-2 out), ACT drain.
  - PtX partials drain to bf16; a tiny warm-up AllReduce issued at t=0
    absorbs the collectives stream's one-time barrier (~50us); the real
    AllReduce runs in TWO D-halves (bf16, 256 KiB each) so phase B on
    half 0 overlaps the half-1 collective.
  Phase B (per row-tile pair, per D-half):
    - corr = P @ (gamma * PtX) with ONE fp8 DoubleRow matmul per tile
      per half (lhsT = resident fp8 P.T, K=256=C).
    - ACT drains PSUM to bf16, DVE adds the exact f32 residual writing
      bf16, one store DMA per tile PAIR; the host upcasts to f32
      (output values are bf16-rounded: ~1e-3 rel err vs the 2e-2 gate).

The host side only reshapes: shards X rows, passes W transposed (pure
relayout, still f32) and b as-is.
"""

import sys

import numpy as np

if "/opt/trn_rl_repo" not in sys.path:
    sys.path.insert(0, "/opt/trn_rl_repo")

N, D, C = 32768, 1024, 256
GAMMA = 1e-4
NCORES = 8
NLOC = N // NCORES  # 4096
P = 128
NT = NLOC // P  # 32
NPAIR = NT // 2  # 16
DH = 512

_cache = {}


def _build_nc():
    import concourse.tile as tile
    from concourse import bacc
    import concourse.mybir as mybir
    from concourse.masks import make_identity
    from contextlib import ExitStack

    f32 = mybir.dt.float32
    bf16 = mybir.dt.bfloat16
    fp8 = mybir.dt.float8e4
    f16 = mybir.dt.float16
    DR = mybir.MatmulPerfMode.DoubleRow
    AF = mybir.ActivationFunctionType

    nc = bacc.Bacc("TRN2", target_bir_lowering=False, debug=False, num_devices=NCORES)
    X = nc.dram_tensor("X", [NLOC, D], f32, kind="ExternalInput").ap()
    Wt = nc.dram_tensor("Wt", [D, C], f32, kind="ExternalInput").ap()
    bvec = nc.dram_tensor("b", [C], f32, kind="ExternalInput").ap()
    out = nc.dram_tensor("out", [NLOC, D], bf16, kind="ExternalOutput").ap()

    with tile.TileContext(nc) as tc, ExitStack() as ctx:
        const = ctx.enter_context(tc.tile_pool(name="const", bufs=1))
        xres = ctx.enter_context(tc.tile_pool(name="xres", bufs=1))
        # xqp holds an fp8 row-tile PAIR; written at load(2p)/load(2p+1),
        # read by transpose and by ptx(pair) ~3 steps later.
        xqp = ctx.enter_context(tc.tile_pool(name="xqp", bufs=3))
        xtp = ctx.enter_context(tc.tile_pool(name="xtp", bufs=2))
        ppool = ctx.enter_context(tc.tile_pool(name="ppool", bufs=4))
        pqp = ctx.enter_context(tc.tile_pool(name="pqp", bufs=3))
        spool = ctx.enter_context(tc.tile_pool(name="spool", bufs=4))
        opool = ctx.enter_context(tc.tile_pool(name="opool", bufs=4))
        cpool = ctx.enter_context(tc.tile_pool(name="cpool", bufs=4))
        dram = ctx.enter_context(tc.tile_pool(name="dram", bufs=1, space="DRAM"))

        Xall = xres.tile([P, NT, D], f32)  # resident f32 X (residual)
        Pt = const.tile([P, 2, NLOC], fp8)  # resident P.T (c-chunk major)

        ident8 = const.tile([P, P], fp8)
        make_identity(nc, ident8)
        ident16 = const.tile([P, P], f16)
        make_identity(nc, ident16)

        # Wq2[p, q, i, c] = W.T[256q + 2p + i, c] in fp8: the DoubleRow
        # K-pair (p, i) matches the fp16-packed transpose output.
        Wq2 = const.tile([P, 4, 2, C], fp8)
        with tc.tile_pool(name="wtmp", bufs=1) as wtmp:
            wt_f = wtmp.tile([P, 4, 2, C], f32)
            nc.sync.dma_start(
                wt_f[:], Wt.rearrange("(q p two) c -> p q two c", p=P, two=2)
            )
            nc.scalar.copy(Wq2[:], wt_f[:])

        ones1 = const.tile([1, P], bf16)
        nc.vector.memset(ones1[:], 1.0)
        b_sb = const.tile([1, C], bf16)
        with tc.tile_pool(name="btmp", bufs=1) as btmp:
            b_f = btmp.tile([1, C], f32)
            nc.sync.dma_start(b_f[:], bvec.rearrange("(o c) -> o c", o=1))
            nc.vector.tensor_copy(b_sb[:], b_f[:])

        ar_in = [dram.tile([C, DH], bf16, name=f"ar_in{h}") for h in range(2)]
        ar_out = [
            dram.tile([C, DH], bf16, addr_space="Shared", name=f"ar_out{h}")
            for h in range(2)
        ]

        # Tiny warm-up AllReduce: absorbs the collectives stream's
        # one-time BARRIER/init concurrently with phase A.
        warm_in = dram.tile([1, 64], f32, name="warm_in")
        warm_out = dram.tile([1, 64], f32, addr_space="Shared", name="warm_out")
        with tc.tile_pool(name="wrm", bufs=1) as wrm:
            w_sb = wrm.tile([1, 64], f32)
            nc.vector.memset(w_sb[:], 0.0)
            nc.sync.dma_start(warm_in[:], w_sb[:])
        nc.gpsimd.collective_compute(
            "AllReduce",
            mybir.AluOpType.add,
            replica_groups=[list(range(NCORES))],
            ins=[warm_in[:].opt()],
            outs=[warm_out[:].opt()],
        )

        # X loads AFTER the small setup DMAs (the Sync queue is in-order:
        # W/b must not queue behind 16 MB of X traffic); two row-tiles
        # per DMA halves the issue count.
        xr = X.rearrange("(t p) d -> p t d", p=P)
        for t in range(NPAIR):
            nc.sync.dma_start(Xall[:, 2 * t:2 * t + 2, :], xr[:, 2 * t:2 * t + 2, :])

        # ---- phase A ----
        def s_cast(i):
            if i % 2 == 0:
                return xqp.tile([P, 2, D], fp8, name="xq", tag="xq")
            return None

        def s_transpose(i, xq):
            # fp8 cast on DVE, then 4 PE transposes of the fp16 view
            # (adjacent-d fp8 pairs); the drain de-interleaves into the
            # standard DR weight layout, alternating ACT/DVE.
            nc.vector.tensor_copy(xq[:, i % 2, :], Xall[:, i, :])
            xt2 = xtp.tile([P, 4, 2, P], fp8, name="xt", tag="xt")
            trp = psA.tile([P, 4, P], f16, name="trp", tag="trp")
            xq16 = xq[:, i % 2, :].bitcast(f16)  # [P, 512]
            for q in range(4):
                nc.tensor.matmul(
                    trp[:, q, :],
                    xq16[:, q * P:(q + 1) * P],
                    ident16[:],
                    is_transpose=True,
                    start=(q == 0),
                    stop=(q == 3),
                )
            src = trp[:].bitcast(fp8).rearrange("p q (n two) -> p q two n", two=2)
            if i % 2 == 0:
                nc.scalar.copy(xt2[:], src)
            else:
                nc.vector.tensor_copy(xt2[:], src)
            return xt2

        def s_logits(i, xt2):
            lg = psL.tile([P, C], f32, name="lg", tag="lg")
            for q in range(4):
                nc.tensor.matmul(
                    lg[:],
                    xt2[:, q, :, :],
                    Wq2[:, q, :, :],
                    start=(q == 0),
                    stop=False,
                    perf_mode=DR,
                )
            nc.tensor.matmul(lg[:], ones1[:], b_sb[:], start=False, stop=True)
            return lg

        def s_softmax(i, lg, pq):
            # |logits| <= ~10 so exp is safe without max-subtraction
            p_sb = ppool.tile([P, C], f32, name="p_sb", tag="p")
            ssum = spool.tile([P, 1], f32, name="ssum", tag="s")
            nc.scalar.activation(p_sb[:], lg[:], AF.Exp, accum_out=ssum[:])
            rinv = spool.tile([P, 1], f32, name="rinv", tag="r")
            nc.vector.reciprocal(rinv[:], ssum[:])
            nc.vector.tensor_scalar_mul(pq[:, i % 2, :], p_sb[:], rinv[:])

        def s_ptx(pair, pq, xq):
            # PtX += P_pair.T @ X_pair, fp8 DoubleRow over K=256 rows
            for cc in range(2):
                for h in range(2):
                    nc.tensor.matmul(
                        ptx_ps[2 * cc + h][:],
                        pq[:, :, cc * P:(cc + 1) * P],
                        xq[:, :, h * DH:(h + 1) * DH],
                        start=(pair == 0),
                        stop=(pair == NPAIR - 1),
                        perf_mode=DR,
                    )
            # P.T for phase B: 4 fp8 transposes (stride-2 out), ACT drain.
            ptp = psP.tile([P, 2, C, 2], fp8, name="ptp", tag="ptp")
            for j in range(2):
                for cc in range(2):
                    nc.tensor.matmul(
                        ptp[:, j, cc * P:(cc + 1) * P, 0],
                        pq[:, j, cc * P:(cc + 1) * P],
                        ident8[:],
                        is_transpose=True,
                        start=(j == 0 and cc == 0),
                        stop=(j == 1 and cc == 1),
                    )
            dst = Pt[:, :, 2 * pair * P:(2 * pair + 2) * P].rearrange(
                "p cc (j r) -> p cc j r", j=2
            )
            src = ptp[:, :, :, 0].rearrange("p j (cc r) -> p cc j r", cc=2)
            nc.scalar.copy(dst, src)

        with tc.tile_pool(name="psA", bufs=1, space="PSUM") as psA, \
             tc.tile_pool(name="psP", bufs=1, space="PSUM") as psP, \
             tc.tile_pool(name="psL", bufs=2, space="PSUM") as psL, \
             tc.tile_pool(name="psX", bufs=1, space="PSUM") as psX:
            ptx_ps = [
                psX.tile([P, DH], f32, name=f"ptx_{c}_{h}", tag=f"ptx_{c}_{h}")
                for c in range(2)
                for h in range(2)
            ]
            # Pipeline: at step i run logits(i), transpose(i+1), and ptx
            # for the pair ending at tile i-2. The 2-step skew lets the
            # ACT exp latency hide under PE work.
            state = {}

            xq0 = s_cast(0)
            state[0] = (xq0, s_transpose(0, xq0), None)
            xq1 = s_cast(1)
            state[1] = (xq1 if xq1 is not None else xq0, None, None)
            for i in range(NT):
                xq_i, xt_i, _ = state[i]
                # transpose(i+1) FIRST: its PSUM drain then overlaps the
                # PE's logits(i)/ptx work instead of stalling logits(i+1)
                if i + 1 < NT:
                    xq_n, _, _ = state[i + 1]
                    state[i + 1] = (xq_n, s_transpose(i + 1, xq_n), None)
                lg = s_logits(i, xt_i)
                if i % 2 == 0:
                    pq = pqp.tile([P, 2, C], fp8, name="pq", tag="pq")
                else:
                    pq = state[i - 1][2]
                s_softmax(i, lg, pq)
                state[i] = (xq_i, xt_i, pq)
                if i + 2 < NT:
                    xq_f = s_cast(i + 2)
                    state[i + 2] = (
                        xq_f if xq_f is not None else state[i + 1][0],
                        None,
                        None,
                    )
                if i >= 3 and (i - 2) % 2 == 1:
                    pair = (i - 3) // 2
                    xq_p = state.pop(2 * pair)[0]
                    pq_p = state.pop(2 * pair + 1)[2]
                    s_ptx(pair, pq_p, xq_p)
            # in-loop ptx covers pairs 0..NPAIR-2; finish the last pair
            for pair in (NPAIR - 1,):
                xq_p = state.pop(2 * pair)[0]
                pq_p = state.pop(2 * pair + 1)[2]
                s_ptx(pair, pq_p, xq_p)

            # PSUM -> SBUF -> DRAM bounce, one per D-half, cast to bf16
            # (the PtX rounding is gamma-scaled: invisible in the output).
            for h in range(2):
                s = const.tile([P, 2, DH], bf16, name=f"stg{h}", tag=f"stg{h}")
                nc.vector.tensor_copy(s[:, 0, :], ptx_ps[h][:])
                nc.scalar.copy(s[:, 1, :], ptx_ps[2 + h][:])
                nc.sync.dma_start(
                    ar_in[h].rearrange("(c p) d -> p c d", p=P), s[:]
                )

        # ---- phase B, interleaved with the collectives: AllReduce h=1 is
        # emitted AFTER phase B h=0 so h=0's consumers only wait on the
        # first collective's completion tick, and the second collective
        # runs concurrently with h=0 compute. gamma folded into PtX. ----
        def ar(h):
            nc.gpsimd.collective_compute(
                "AllReduce",
                mybir.AluOpType.add,
                replica_groups=[list(range(NCORES))],
                ins=[ar_in[h][:].opt()],
                outs=[ar_out[h][:].opt()],
            )

        def phase_b(h, psB):
            pall = const.tile([P, 2, DH], bf16, name=f"pall{h}", tag=f"stg{h}")
            nc.sync.dma_start(
                pall[:], ar_out[h].rearrange("(c p) d -> p c d", p=P)
            )
            ptxq = const.tile([P, 2, DH], fp8, name=f"ptxq{h}")
            nc.vector.tensor_scalar_mul(ptxq[:], pall[:], GAMMA)
            o_dst = out[:, h * DH:(h + 1) * DH].rearrange("(t p) d -> p t d", p=P)
            for t in range(NPAIR):
                o2 = opool.tile([P, 2, DH], bf16, name="o2", tag="o")
                cor2 = psB.tile([P, 2, DH], f32, name="cor2", tag="cor")
                for j in range(2):
                    i = 2 * t + j
                    nc.tensor.matmul(
                        cor2[:, j, :],
                        Pt[:, :, i * P:(i + 1) * P],
                        ptxq[:],
                        start=True,
                        stop=True,
                        perf_mode=DR,
                        skip_group_check=True,
                    )
                # pair-wise drain + residual add (one op each per pair)
                cors = cpool.tile([P, 2, DH], bf16, name="cors", tag="cs")
                nc.scalar.copy(cors[:], cor2[:])
                nc.vector.tensor_add(
                    o2[:], cors[:], Xall[:, 2 * t:2 * t + 2, h * DH:(h + 1) * DH]
                )
                nc.sync.dma_start(o_dst[:, 2 * t:2 * t + 2, :], o2[:])

        with tc.tile_pool(name="psB", bufs=4, space="PSUM") as psB:
            ar(0)
            phase_b(0, psB)
            ar(1)
            phase_b(1, psB)

    nc.finalize()
    return nc


def _run(inputs, trace=False, **kwargs):
    from concourse import bass_utils

    if "nc" not in _cache:
        _cache["nc"] = _build_nc()
    nc = _cache["nc"]

    X = np.ascontiguousarray(np.asarray(inputs["X"], dtype=np.float32))
    W = np.ascontiguousarray(np.asarray(inputs["W"], dtype=np.float32))
    b = np.ascontiguousarray(np.asarray(inputs["b"], dtype=np.float32))
    Wt = np.ascontiguousarray(W.T)

    in_maps = [
        {"X": X[i * NLOC:(i + 1) * NLOC], "Wt": Wt, "b": b} for i in range(NCORES)
    ]
    res = bass_utils.run_bass_kernel_spmd(
        nc, in_maps, core_ids=list(range(NCORES)), trace=trace, **kwargs
    )
    outp = np.concatenate(
        [np.asarray(res.results[i]["out"]).astype(np.float32) for i in range(NCORES)],
        axis=0,
    )
    return outp, res


def kernel(**inputs):
    outp, _ = _run(inputs, trace=False)
    return outp
